# revision 22
# baseline (speedup 1.0000x reference)
"""MultiHeadAttention TRN2 Bass kernel, sharded over 8 NeuronCores.

Sharding: 8 cores = 2 batches x 4 head-groups. Each core computes 4 heads of
one batch end-to-end (q/k/v projections, biased+masked softmax attention, and
a partial output projection); the host sums the per-group partial outputs.

v3 design (single interleaved stream, ScalarE exp is the critical path):
  - all-bf16 matmuls (fp8 measured too lossy: attention does not average away
    per-element weight noise, so fp8's ~3.6% RMS passes straight to the
    output). K=128 everywhere - scores use per-head zero-padded kT tiles so
    the PE never switches tiling mode (mode switches drain the array and
    serialize weight loads).
  - exp on ScalarE with scale=1/8 (the 1/sqrt(dh)) and bias=-6*ln2 (harmless
    constant shift; cancels in the softmax normalize). 128 activations of
    [128,1024] = the ~145us critical path; ScalarE does nothing else.
  - a = e * expb (mask*exp(chem_bias), bf16) on DVE in 2x mode; attn@v with
    a ones-column denominator (even heads: ones col 64; odd heads: ones col
    0, v dims in 64:128 so every epilogue op stays partition-aligned).
  - out2 is evacuated PSUM->SBUF so the single PSUM accumulator slot frees
    for the next head; the normalize epilogue (den row -> DRAM -> [128,8]
    spread -> reciprocal -> broadcast -> scale + v-bias) is software-
    pipelined into the next head's chunk stream.
  - x tiles stream through a shared 4-slot pool; the projection units that
    consume late-arriving halves (k/v second half, q superblock 1) are
    injected into the first attention head's chunk stream, and the output
    projection (bf16 partials, summed on host) into the second superblock's.
"""

import numpy as np
import ml_dtypes

import concourse.bass as bass
import concourse.mybir as mybir
import concourse.tile as tile
from concourse.bacc import Bacc

BF16 = mybir.dt.bfloat16
F32 = mybir.dt.float32
nbf16 = ml_dtypes.bfloat16

B = 2
S = 2048
D = 1024
H = 16
DH = 64
HPC = 4  # heads per core
CD = HPC * DH  # 256 per-core projected dims
NCORES = 8

KC = D // 128  # 8 contraction chunks for projections
TC = S // 128  # 16 token (s_k) chunks
SUPS = 2
SUPLEN = S // SUPS  # 1024 columns per s_q superblock
NB = 512  # projection/outproj token block

EXP_SCALE = 0.125  # 1/sqrt(dh)
EXP_BIAS = -4.158883083359672  # -6*ln2, cancels in the normalize


def build_module(debug=False):
    nc = Bacc(None)

    xq_d = nc.dram_tensor("xq", [128, KC, S], BF16, kind="ExternalInput")
    xk_d = nc.dram_tensor("xk", [128, KC, S], BF16, kind="ExternalInput")
    xv_d = nc.dram_tensor("xv", [128, KC, S], BF16, kind="ExternalInput")
    wq_d = nc.dram_tensor("wq", [128, KC, CD], BF16, kind="ExternalInput")
    wk_d = nc.dram_tensor("wk", [128, KC, CD], BF16, kind="ExternalInput")
    wv_d = nc.dram_tensor("wv", [128, KC, CD], BF16, kind="ExternalInput")
    wo_d = nc.dram_tensor("wo", [128, CD // 128, D], BF16, kind="ExternalInput")
    bq_d = nc.dram_tensor("bq", [128, 2], F32, kind="ExternalInput")
    bk_d = nc.dram_tensor("bk", [128, 2], F32, kind="ExternalInput")
    bv_d = nc.dram_tensor("bv", [128, 2], F32, kind="ExternalInput")
    expb_d = nc.dram_tensor("expb", [S, S], BF16, kind="ExternalInput")  # [s_k, s_q]
    pout_d = nc.dram_tensor("pout", [D, S], BF16, kind="ExternalOutput")
    if debug:
        dbg_qT = nc.dram_tensor("dbg_qT", [2, 128, S], BF16, kind="ExternalOutput")
        dbg_kT = nc.dram_tensor("dbg_kT", [HPC, 128, S], BF16, kind="ExternalOutput")
        dbg_vv = nc.dram_tensor("dbg_vv", [128, HPC * TC * 128], BF16, kind="ExternalOutput")
        dbg_cc = nc.dram_tensor("dbg_cc", [2, 128, S], BF16, kind="ExternalOutput")

    with tile.TileContext(nc) as tc:
        with (
            tc.tile_pool(name="statics", bufs=1) as statics,
            tc.tile_pool(name="xh", bufs=4) as x_pool,
            tc.tile_pool(name="expb", bufs=4) as expb_pool,
            tc.tile_pool(name="e", bufs=3) as e_pool,
            tc.tile_pool(name="a", bufs=10) as a_pool,
            tc.tile_pool(name="o2s", bufs=2) as o2s_pool,
            tc.tile_pool(name="spr", bufs=2) as spr_pool,
            tc.tile_pool(name="rb", bufs=2) as rb_pool,
            tc.tile_pool(name="oev", bufs=2) as oev_pool,
            tc.tile_pool(name="psc", bufs=2, space="PSUM") as psc,
            tc.tile_pool(name="pacc", bufs=1, space="PSUM") as pacc,
            tc.tile_pool(name="pop", bufs=2, space="PSUM") as pop,
            tc.tile_pool(name="dsc", bufs=4, space="DRAM") as dram_pool,
        ):
            # ---- statics ----
            wq_sb = statics.tile([128, KC, CD], BF16, name="wq_sb")
            wk_sb = statics.tile([128, KC, CD], BF16, name="wk_sb")
            wv_sb = statics.tile([128, KC, CD], BF16, name="wv_sb")
            wo_sb = statics.tile([128, CD // 128, D], BF16, name="wo_sb")
            bq_sb = statics.tile([128, 2], F32, name="bq_sb")
            bk_sb = statics.tile([128, 2], F32, name="bk_sb")
            bv_sb = statics.tile([128, 2], F32, name="bv_sb")
            bias_t = statics.tile([128, 1], F32, name="bias_t")
            qT = [statics.tile([128, S], BF16, name=f"qT{m}") for m in range(2)]
            # per-head kT, zero-padded on the other head's 64 rows so every
            # scores matmul contracts a full K=128 (no PE mode switches)
            kTh = [statics.tile([128, S], BF16, name=f"kTh{h}") for h in range(HPC)]
            cc = [statics.tile([128, S], BF16, name=f"cc{m}") for m in range(2)]
            # vv[:, vh, tk, :]: attnv lhsT per (head, s_k chunk), heads in vh
            # order (0,2,1,3; Wv host-reordered). Even-parity heads: v dims in
            # cols 0:64, ones col 64; odd parity: ones col 0, v dims 64:128.
            vv = statics.tile([128, HPC, TC, 128], BF16, name="vv")

            # memsets on DVE (idle until the first projection evac)
            nc.vector.memset(bias_t, EXP_BIAS)
            for h in range(HPC):
                nc.vector.memset(kTh[h], 0.0)
            nc.vector.memset(vv, 0.0)
            nc.vector.memset(vv[:, 0:2, :, 64:65], 1.0)
            nc.vector.memset(vv[:, 2:4, :, 0:1], 1.0)

            # ---- input loads, spread across the three DGE-capable queues ----
            # x half-tiles stream through a 4-slot pool; alloc order matters:
            # [xq0, xk0, xv0, xk1] then xq1 -> xq0's slot, xv1 -> xk0's slot.
            def x_half(src, half, eng, split=2):
                t = x_pool.tile([128, KC, SUPLEN], BF16, name="xh")
                for q in range(split):  # first-needed column block lands first
                    w = SUPLEN // split
                    qsl_s = slice(half * SUPLEN + q * w, half * SUPLEN + (q + 1) * w)
                    qsl_d = slice(q * w, (q + 1) * w)
                    for kc in range(KC):
                        eng.dma_start(t[:, kc, qsl_d], src[:, kc, qsl_s])
                return t

            # scalar queue is free until the first exp (~13us): it carries xq0
            xq0 = x_half(xq_d, 0, nc.scalar)

            nc.sync.dma_start(bq_sb, bq_d[:, :])
            nc.sync.dma_start(bk_sb, bk_d[:, :])
            nc.sync.dma_start(bv_sb, bv_d[:, :])
            for j in range(4):
                nc.sync.dma_start(wq_sb[:, 2 * j : 2 * j + 2, :], wq_d[:, 2 * j : 2 * j + 2, :])
            for j in range(4):
                nc.sync.dma_start(wk_sb[:, 2 * j : 2 * j + 2, :], wk_d[:, 2 * j : 2 * j + 2, :])
            xk0 = x_half(xk_d, 0, nc.sync)
            for j in range(4):
                nc.sync.dma_start(wv_sb[:, 2 * j : 2 * j + 2, :], wv_d[:, 2 * j : 2 * j + 2, :])

            # ---- expb streaming (gpsimd queue): tiles of 4 s_k chunks ----
            expb_tiles = [None] * 8

            def emit_expb(t):
                tl = expb_pool.tile([128, 4, SUPLEN], BF16, name="expb")
                sup, g = divmod(t, 4)
                src = expb_d[:, sup * SUPLEN : (sup + 1) * SUPLEN].rearrange(
                    "(c p) q -> p c q", p=128
                )
                for j in range(4):
                    for hf in range(2):
                        nc.gpsimd.dma_start(
                            tl[:, j, hf * NB : (hf + 1) * NB],
                            src[:, g * 4 + j, hf * NB : (hf + 1) * NB],
                        )
                expb_tiles[t] = tl

            emit_expb(0)
            emit_expb(1)
            xv0 = x_half(xv_d, 0, nc.gpsimd)
            emit_expb(2)
            xk1 = x_half(xk_d, 1, nc.sync)
            emit_expb(3)
            xv1 = x_half(xv_d, 1, nc.gpsimd)
            xq1 = x_half(xq_d, 1, nc.gpsimd, split=1)
            nc.gpsimd.dma_start(wo_sb, wo_d[:, :, :])

            # ---- projection units ----
            def qk_proj(xt, half, w_sb, b_sb, dst, nt):
                # nt is the global token block; xt holds columns of `half`
                lsl = slice(nt * NB - half * SUPLEN, (nt + 1) * NB - half * SUPLEN)
                csl = slice(nt * NB, (nt + 1) * NB)
                for mt in range(2):
                    ps = pop.tile([128, NB], F32, name="ps_p", tag="pop")
                    for kc in range(KC):
                        nc.tensor.matmul(
                            ps,
                            lhsT=w_sb[:, kc, mt * 128 : (mt + 1) * 128],
                            rhs=xt[:, kc, lsl],
                            start=(kc == 0),
                            stop=(kc == KC - 1),
                        )
                    if dst is qT:
                        nc.vector.tensor_scalar_add(
                            qT[mt][:, csl], ps, scalar1=b_sb[:, mt : mt + 1]
                        )
                    else:  # split into zero-padded per-head kT tiles
                        h0, h1 = 2 * mt, 2 * mt + 1
                        nc.vector.tensor_scalar_add(
                            kTh[h0][0:64, csl], ps[0:64, :],
                            scalar1=b_sb[0:64, mt : mt + 1],
                        )
                        nc.vector.tensor_scalar_add(
                            kTh[h1][64:128, csl], ps[64:128, :],
                            scalar1=b_sb[64:128, mt : mt + 1],
                        )

            def v_proj(xt, tk):
                # xt holds the half containing s_k chunk tk
                lsl = slice((tk % 8) * 128, (tk % 8 + 1) * 128)
                ps = pop.tile([128, CD], F32, name="ps_v", tag="pop")
                for kc in range(KC):
                    nc.tensor.matmul(
                        ps,
                        lhsT=xt[:, kc, lsl],
                        rhs=wv_sb[:, kc, :],
                        start=(kc == 0),
                        stop=(kc == KC - 1),
                    )
                psh = ps.rearrange("p (h d) -> p h d", h=HPC)
                # Wv host-reordered to vh order (0,2,1,3): first two blocks are
                # the even-parity heads (cols 0:64), last two odd (cols 64:128)
                nc.vector.tensor_copy(vv[:, 0:2, tk, 0:DH], psh[:, 0:2, :])
                nc.vector.tensor_copy(vv[:, 2:4, tk, DH:128], psh[:, 2:4, :])

            # prefix: q sup0 and k first half (all the first exp needs)
            for nt in range(2):
                qk_proj(xq0, 0, wq_sb, bq_sb, qT, nt)
            for nt in range(2):
                qk_proj(xk0, 0, wk_sb, bk_sb, kTh, nt)

            # everything else is deferred, injected into the attention stream
            # as its DMAs land (the input load is DMA-bandwidth-bound early on)
            deferred = (
                [("k", xk1, 2), ("k", xk1, 3)]
                + [("v", xv0, tk) for tk in range(8)]
                + [("v", xv1, tk) for tk in range(8, 16)]
            )
            dptr = [0]

            def emit_deferred(n=1):
                for _ in range(n):
                    if dptr[0] >= len(deferred):
                        return
                    kind, xt, idx = deferred[dptr[0]]
                    dptr[0] += 1
                    if kind == "k":
                        qk_proj(xt, 1, wk_sb, bk_sb, kTh, idx)
                    elif kind == "q":
                        qk_proj(xt, 1, wq_sb, bq_sb, qT, idx)
                    else:
                        v_proj(xt, idx)

            # ---- outproj ----
            outproj_queue = [(mo, nt) for nt in range(2) for mo in range(D // 128)]
            outproj_queue += [(mo, nt) for nt in range(2, 4) for mo in range(D // 128)]
            op_cursor = [0]
            OP_SUP0_TILES = 16

            def emit_outproj(limit, n=1):
                for _ in range(n):
                    if op_cursor[0] >= limit:
                        return
                    mo, nt = outproj_queue[op_cursor[0]]
                    op_cursor[0] += 1
                    csl = slice(nt * NB, (nt + 1) * NB)
                    ps = pop.tile([128, NB], F32, name="ps_o", tag="pop")
                    for kc in range(2):
                        nc.tensor.matmul(
                            ps,
                            lhsT=wo_sb[:, kc, mo * 128 : (mo + 1) * 128],
                            rhs=cc[kc][:, csl],
                            start=(kc == 0),
                            stop=(kc == 1),
                        )
                    ot = oev_pool.tile([128, NB], BF16, name="ot")
                    nc.vector.tensor_copy(ot, ps)
                    nc.sync.dma_start(pout_d[mo * 128 : (mo + 1) * 128, csl], ot)

            # ---- epilogue (3 stages, pipelined into the next head) ----
            def make_epilogue(sup, h, o2s):
                mt, hh = h // 2, h % 2
                prow = slice(hh * 64, (hh + 1) * 64)
                den = 64 if hh == 0 else 0
                qsl = slice(sup * SUPLEN, (sup + 1) * SUPLEN)
                st = {}

                def s1():
                    # den row -> DRAM -> [128, 8] spread (wide reciprocal)
                    rsd = dram_pool.tile([1, SUPLEN], F32, name="rsd")
                    nc.sync.dma_start(rsd, o2s[den : den + 1, :])
                    spread = spr_pool.tile([128, SUPLEN // 128], F32, name="spread")
                    nc.sync.dma_start(
                        spread, rsd[:, :].rearrange("a (p f) -> (a p) f", p=128)
                    )
                    st["spread"] = spread

                def s2():
                    nc.vector.reciprocal(st["spread"], st["spread"])
                    rsd2 = dram_pool.tile([1, SUPLEN], F32, name="rsd2")
                    nc.sync.dma_start(
                        rsd2[:, :].rearrange("a (p f) -> (a p) f", p=128), st["spread"]
                    )
                    rbt = rb_pool.tile([128, SUPLEN], F32, name="rbt")
                    nc.sync.dma_start(rbt[prow, :], rsd2[:, :].partition_broadcast(64))
                    st["rbt"] = rbt

                def s3():
                    seg = cc[mt][prow, qsl]
                    nc.vector.tensor_mul(seg, o2s[prow, :], st["rbt"][prow, :])
                    nc.vector.tensor_scalar_add(
                        seg, seg, scalar1=bv_sb[prow, mt : mt + 1]
                    )

                return [s1, s2, s3]

            # ---- attention ----
            pending = None
            for sup in range(SUPS):
                for h in range(HPC):
                    mt, hh = h // 2, h % 2
                    vh = {0: 0, 2: 1, 1: 2, 3: 3}[h]
                    out2 = pacc.tile([128, SUPLEN], F32, name="out2", tag="pacc")
                    a_tiles = [None] * TC
                    lag = 8 if (sup == 0 and h == 0) else 2

                    def attnv(ck):
                        for hf in range(2):
                            hsl = slice(hf * NB, (hf + 1) * NB)
                            nc.tensor.matmul(
                                out2[:, hsl],
                                lhsT=vv[:, vh, ck, :],
                                rhs=a_tiles[ck][:, hsl],
                                start=(ck == 0),
                                stop=(ck == TC - 1),
                            )
                        a_tiles[ck] = None

                    for ck in range(TC):
                        if sup == 0 and h == 3 and ck in (0, 4, 8, 12):
                            emit_expb(4 + ck // 4)  # prefetch sup1 expb
                        t = sup * 4 + ck // 4
                        sc = psc.tile([128, SUPLEN], F32, name="sc", tag="psc")
                        lhsT_k = kTh[h][:, ck * 128 : (ck + 1) * 128]
                        for hf in range(2):
                            hsl = slice(hf * NB, (hf + 1) * NB)
                            nc.tensor.matmul(
                                sc[:, hsl],
                                lhsT=lhsT_k,
                                rhs=qT[mt][:, sup * SUPLEN + hf * NB : sup * SUPLEN + (hf + 1) * NB],
                                start=True,
                                stop=True,
                            )
                        e = e_pool.tile([128, SUPLEN], BF16, name="e")
                        nc.scalar.activation(
                            e, sc, func=mybir.ActivationFunctionType.Exp,
                            bias=bias_t[:, 0:1], scale=EXP_SCALE,
                        )
                        a = a_pool.tile([128, SUPLEN], BF16, name="a")
                        nc.vector.tensor_mul(a, e, expb_tiles[t][:, ck % 4, :])
                        a_tiles[ck] = a
                        # deferred projection units ride the h0 stream (before
                        # this chunk's lagged attnv, which may consume them)
                        if sup == 0 and h == 0 and ck >= 6:
                            emit_deferred(1 if ck < 8 else 2)
                        if sup == 0 and h == 2 and ck in (1, 3):
                            # q sup1 (its x half lands late; needed by sup1)
                            qk_proj(xq1, 1, wq_sb, bq_sb, qT, ck // 2 + 2)
                        if ck >= lag:
                            attnv(ck - lag)
                        # previous head's epilogue
                        if pending is not None:
                            if ck == 3:
                                pending[0]()
                            elif ck == 6:
                                pending[1]()
                            elif ck == 9:
                                pending[2]()
                                pending = None
                        # outproj interleave during sup1
                        if sup == 1 and ck in (5, 10, 15) and (h > 0 or ck > 9):
                            emit_outproj(OP_SUP0_TILES, 2)
                    for ck in range(TC - lag, TC):
                        attnv(ck)
                    # evacuate out2 so the PSUM slot frees for the next head
                    o2s = o2s_pool.tile([128, SUPLEN], F32, name="o2s")
                    nc.vector.tensor_copy(o2s, out2)
                    if pending is not None:
                        for f in pending:
                            f()
                    pending = make_epilogue(sup, h, o2s)

            # tail: final epilogue + remaining outproj tiles
            for f in pending:
                f()
            pending = None
            emit_outproj(len(outproj_queue), len(outproj_queue))

            if debug:
                for m in range(2):
                    nc.sync.dma_start(dbg_qT[m, :, :], qT[m])
                    nc.sync.dma_start(dbg_cc[m, :, :], cc[m])
                for h in range(HPC):
                    nc.sync.dma_start(dbg_kT[h, :, :], kTh[h])
                nc.sync.dma_start(dbg_vv[:, :], vv.rearrange("p a b c -> p (a b c)"))

    nc.finalize()
    return nc


def make_in_maps(query, key, value, mask, chemical_bias, Wq, bq, Wk, bk, Wv, bv, Wo):
    """Host-side preprocessing: per-core input dicts (8 cores)."""
    f32 = np.float32

    def xarr(x):
        # [S, D] -> [128, KC, S]: arr[p, kc, s] = x[s, kc*128+p]
        return np.ascontiguousarray(
            np.asarray(x, f32).T.reshape(KC, 128, S).transpose(1, 0, 2)
        ).astype(nbf16)

    per_batch = []
    for b in range(B):
        xq = xarr(query[b])
        xk = xarr(key[b])
        xv = xarr(value[b])
        bm = np.where(mask[b, 0] == 0, f32(0.0), np.exp(chemical_bias[b], dtype=f32))
        expbT = np.ascontiguousarray(bm.T, dtype=nbf16)  # [s_k, s_q]
        per_batch.append((xq, xk, xv, expbT))

    def warr(wt):
        # [D, CD] -> [128, KC, CD]
        return np.ascontiguousarray(
            np.asarray(wt, f32).reshape(KC, 128, CD).transpose(1, 0, 2)
        ).astype(nbf16)

    per_group = []
    for g in range(4):
        hsl = slice(g * CD, (g + 1) * CD)
        wq_ = warr(Wq[hsl].T)
        wk_ = warr(Wk[hsl].T)
        # Wv columns reordered to vh head order (0,2,1,3) for contiguous evacs
        wv_full = np.asarray(Wv[hsl].T, f32).reshape(D, HPC, DH)
        wv_ = warr(np.ascontiguousarray(wv_full[:, [0, 2, 1, 3], :]).reshape(D, CD))
        wo_ = np.ascontiguousarray(
            np.asarray(Wo[:, hsl].T, f32).reshape(2, 128, D).transpose(1, 0, 2)
        ).astype(nbf16)
        bq_ = np.ascontiguousarray(np.asarray(bq[hsl], f32).reshape(2, 128).T)
        bk_ = np.ascontiguousarray(np.asarray(bk[hsl], f32).reshape(2, 128).T)
        bv_ = np.ascontiguousarray(np.asarray(bv[hsl], f32).reshape(2, 128).T)
        per_group.append((wq_, wk_, wv_, wo_, bq_, bk_, bv_))

    in_maps = []
    for core in range(NCORES):
        b, g = divmod(core, 4)
        xq, xk, xv, expbT = per_batch[b]
        wq_, wk_, wv_, wo_, bq_, bk_, bv_ = per_group[g]
        in_maps.append(
            {
                "xq": xq, "xk": xk, "xv": xv,
                "wq": wq_, "wk": wk_, "wv": wv_, "wo": wo_,
                "bq": bq_, "bk": bk_, "bv": bv_,
                "expb": expbT,
            }
        )
    return in_maps


def combine_outputs(results, bo):
    """Sum per-group transposed bf16 partials into the full [B, S, D] output."""
    out = np.empty((B, S, D), np.float32)
    for b in range(B):
        acc = results[4 * b]["pout"].astype(np.float32)
        for g in range(1, 4):
            acc = acc + results[4 * b + g]["pout"].astype(np.float32)
        out[b] = acc.T + np.asarray(bo, np.float32)
    return out


_NC_CACHE = {}


def _get_module(debug=False):
    if debug not in _NC_CACHE:
        _NC_CACHE[debug] = build_module(debug=debug)
    return _NC_CACHE[debug]


def run_spmd(in_maps, debug=False, **kwargs):
    from concourse.bass_utils import run_bass_kernel_spmd

    nc = _get_module(debug)
    return run_bass_kernel_spmd(nc, in_maps, core_ids=list(range(NCORES)), **kwargs)


def kernel(query, key, value, mask, chemical_bias, Wq, bq, Wk, bk, Wv, bv, Wo, bo):
    in_maps = make_in_maps(
        query, key, value, mask, chemical_bias, Wq, bq, Wk, bk, Wv, bv, Wo
    )
    res = run_spmd(in_maps)
    return combine_outputs(res.results, bo)
